# revision 1
# baseline (speedup 1.0000x reference)
"""CenterLoss Trainium2 kernel (Bass/Tile, 8 NeuronCores, data-parallel).

loss = (sum_b clip(||y_b - centers[labels_b]||^2, 1e-12, 1e12)
        + B*(C-1)*1e-12) / B * loss_weight

The masked distmat in the reference reduces to a per-row gather: off-label
entries of distmat*mask are exactly 0.0 and clip to 1e-12 each (a closed-form
constant); on-label entries are squared distances ~O(2*D), far above both clip
bounds, so clip never binds on them.

Per core (B/8 = 4096 rows):
  - y shard loaded as [128, 32*128] f32, partition p holds rows p*32..p*32+31
    (contiguous 16 KiB per partition -> max DMA efficiency)
  - centers rows gathered from HBM via SWDGE dma_gather (512 B/row); gather
    order host-permuted so gathered rows land aligned with the y layout
  - DVE: e = y - g ; ACT: Square(e) with accum_out -> per-partition partials
  - GPSIMD partition_all_reduce -> scalar partial per core
Host: sum 8 partials, add eps constant, /B, *loss_weight.
"""

import numpy as np

B = 32768
D = 128
C = 1000
NCORES = 8
BSH = B // NCORES            # 4096 rows per core
P = 128                      # SBUF partitions
RPP = BSH // P               # 32 rows per partition
T_CH = 8                     # t-blocks per pipeline chunk
NCH = RPP // T_CH            # 4 chunks
CHUNK_IDX = P * T_CH         # 1024 gathered rows per chunk
CHUNK_F = T_CH * D           # free-dim elems per chunk (1024)

_CACHE = {}
TRACE = False                # test.py may set kernel.TRACE = True
LAST_RESULTS = None          # BassKernelResults of the last run (for profiling)


def _build():
    import concourse.bacc as bacc
    import concourse.mybir as mybir
    import concourse.tile as tile
    from concourse import bass_isa

    nc = bacc.Bacc("TRN2", target_bir_lowering=False, debug=False)

    y_in = nc.dram_tensor("y", [BSH, D], mybir.dt.float32, kind="ExternalInput")
    idx_in = nc.dram_tensor("idx", [P, BSH // 16], mybir.dt.int16,
                            kind="ExternalInput")
    cen_in = nc.dram_tensor("centers", [C, D], mybir.dt.float32,
                            kind="ExternalInput")
    out = nc.dram_tensor("out", [1, 1], mybir.dt.float32, kind="ExternalOutput")

    # partition p <- rows [p*RPP, (p+1)*RPP), 16 KiB contiguous per partition
    y_view = y_in.ap().rearrange("(p r) d -> p (r d)", p=P)

    with tile.TileContext(nc) as tc:
        with (
            tc.tile_pool(name="io", bufs=1) as io_pool,
            tc.tile_pool(name="yb", bufs=3) as y_pool,
            tc.tile_pool(name="gb", bufs=3) as g_pool,
            tc.tile_pool(name="eb", bufs=3) as e_pool,
            tc.tile_pool(name="sq", bufs=2) as sq_pool,
        ):
            idx_t = io_pool.tile([P, BSH // 16], mybir.dt.int16)
            nc.sync.dma_start(idx_t[:], idx_in[:, :])
            partials = io_pool.tile([P, NCH], mybir.dt.float32)

            for j in range(NCH):
                yt = y_pool.tile([P, CHUNK_F], mybir.dt.float32, tag="yt")
                nc.sync.dma_start(yt[:], y_view[:, j * CHUNK_F:(j + 1) * CHUNK_F])

                gt = g_pool.tile([P, CHUNK_F], mybir.dt.float32, tag="gt")
                nc.gpsimd.dma_gather(
                    gt[:].rearrange("p (t d) -> p t d", d=D),
                    cen_in[:, :],
                    idx_t[:, j * (CHUNK_IDX // 16):(j + 1) * (CHUNK_IDX // 16)],
                    CHUNK_IDX,
                    CHUNK_IDX,
                    D,
                )

                et = e_pool.tile([P, CHUNK_F], mybir.dt.float32, tag="et")
                nc.vector.tensor_sub(et[:], yt[:], gt[:])

                sqt = sq_pool.tile([P, CHUNK_F], mybir.dt.float32, tag="sqt")
                nc.scalar.activation(
                    sqt[:], et[:], mybir.ActivationFunctionType.Square,
                    accum_out=partials[:, j:j + 1],
                )

            colsum = io_pool.tile([P, 1], mybir.dt.float32)
            nc.vector.tensor_reduce(
                colsum[:], partials[:], axis=mybir.AxisListType.X,
                op=mybir.AluOpType.add,
            )
            allred = io_pool.tile([P, 1], mybir.dt.float32)
            nc.gpsimd.partition_all_reduce(
                allred[:], colsum[:], P, bass_isa.ReduceOp.add,
            )
            nc.sync.dma_start(out[:, :], allred[0:1, 0:1])

    nc.compile()
    return nc


def _get_nc():
    if "nc" not in _CACHE:
        _CACHE["nc"] = _build()
    return _CACHE["nc"]


def _make_idx_tile(labels_shard: np.ndarray) -> np.ndarray:
    """int16 SBUF index tile [128, BSH/16] for dma_gather.

    Gather position i writes to SBUF [i%128, i//128]; we want position
    i = t*128 + p to hold centers[labels[p*RPP + t]] so it aligns with the
    y layout (partition p holds rows p*RPP..p*RPP+RPP-1).  The HW reads
    index for position i at [i%16, i//16] of the idx tile (first 16
    partitions, replicated x8 across the Q7 cores).
    """
    lab = np.asarray(labels_shard).astype(np.int16).reshape(P, RPP)
    idx_global = lab.T.reshape(-1)                    # [t*128+p] = labels[p*RPP+t]
    tile16 = idx_global.reshape(BSH // 16, 16).T      # [q, s] = idx_global[s*16+q]
    return np.ascontiguousarray(np.tile(tile16, (P // 16, 1)))


def kernel(y, labels, centers, loss_weight):
    global LAST_RESULTS
    from concourse.bass_utils import run_bass_kernel_spmd

    y = np.asarray(y, dtype=np.float32)
    labels = np.asarray(labels)
    centers = np.ascontiguousarray(np.asarray(centers, dtype=np.float32))

    nc = _get_nc()

    in_maps = []
    for c in range(NCORES):
        sl = slice(c * BSH, (c + 1) * BSH)
        in_maps.append({
            "y": np.ascontiguousarray(y[sl]),
            "idx": _make_idx_tile(labels[sl]),
            "centers": centers,
        })

    res = run_bass_kernel_spmd(
        nc, in_maps, core_ids=list(range(NCORES)), trace=TRACE,
    )
    LAST_RESULTS = res

    total = sum(float(r["out"][0, 0]) for r in res.results)
    total += B * (C - 1) * 1e-12
    loss = total / B * float(np.asarray(loss_weight))
    return np.float32(loss)


# revision 3
# speedup vs baseline: 1.2032x; 1.2032x over previous
"""CenterLoss Trainium2 kernel (Bass/Tile, 8 NeuronCores, data-parallel).

loss = (sum_b clip(||y_b - centers[labels_b]||^2, 1e-12, 1e12)
        + B*(C-1)*1e-12) / B * loss_weight

The masked distmat in the reference reduces to a per-row gather: off-label
entries of distmat*mask are exactly 0.0 and clip to 1e-12 each (a closed-form
constant); on-label entries are squared distances ~O(2*D), far above both clip
bounds, so clip never binds on them.

Per core (B/8 = 4096 rows):
  - y shard loaded as [128, 32*128] f32, partition p holds rows p*32..p*32+31
    (contiguous 16 KiB per partition -> max DMA efficiency)
  - centers rows gathered from HBM via SWDGE dma_gather (512 B/row); gather
    order host-permuted so gathered rows land aligned with the y layout
  - DVE: e = y - g ; ACT: Square(e) with accum_out -> per-partition partials
  - GPSIMD partition_all_reduce -> scalar partial per core
Host: sum 8 partials, add eps constant, /B, *loss_weight.
"""

import numpy as np

B = 32768
D = 128
C = 1000
NCORES = 8
BSH = B // NCORES            # 4096 rows per core
P = 128                      # SBUF partitions
RPP = BSH // P               # 32 rows per partition
T_CH = 8                     # t-blocks per pipeline chunk
NCH = RPP // T_CH            # 4 chunks
CHUNK_IDX = P * T_CH         # 1024 gathered rows per chunk
CHUNK_F = T_CH * D           # free-dim elems per chunk (1024)

_CACHE = {}
TRACE = False                # test.py may set kernel.TRACE = True
LAST_RESULTS = None          # BassKernelResults of the last run (for profiling)


def _build():
    import concourse.bacc as bacc
    import concourse.mybir as mybir
    import concourse.tile as tile
    from concourse import bass_isa

    nc = bacc.Bacc("TRN2", target_bir_lowering=False, debug=False,
                   enable_partition_id=False, num_swdge_queues=4)

    y_in = nc.dram_tensor("y", [BSH, D], mybir.dt.float32, kind="ExternalInput")
    idx_in = nc.dram_tensor("idx", [P, BSH // 16], mybir.dt.int16,
                            kind="ExternalInput")
    cen_in = nc.dram_tensor("centers", [C, D], mybir.dt.float32,
                            kind="ExternalInput")
    out = nc.dram_tensor("out", [1, 1], mybir.dt.float32, kind="ExternalOutput")

    # partition p <- rows [p*RPP, (p+1)*RPP), 16 KiB contiguous per partition
    y_view = y_in.ap().rearrange("(p r) d -> p (r d)", p=P)

    with tile.TileContext(nc) as tc:
        with (
            tc.tile_pool(name="io", bufs=1) as io_pool,
            tc.tile_pool(name="yb", bufs=3) as y_pool,
            tc.tile_pool(name="gb", bufs=3) as g_pool,
            tc.tile_pool(name="eb", bufs=3) as e_pool,
            tc.tile_pool(name="sq", bufs=2) as sq_pool,
        ):
            idx_t = io_pool.tile([P, BSH // 16], mybir.dt.int16)
            nc.sync.dma_start(idx_t[:], idx_in[:, :])
            partials = io_pool.tile([P, NCH], mybir.dt.float32)

            for j in range(NCH):
                yt = y_pool.tile([P, CHUNK_F], mybir.dt.float32, tag="yt")
                nc.sync.dma_start(yt[:], y_view[:, j * CHUNK_F:(j + 1) * CHUNK_F])

                gt = g_pool.tile([P, CHUNK_F], mybir.dt.float32, tag="gt")
                nc.gpsimd.dma_gather(
                    gt[:].rearrange("p (t d) -> p t d", d=D),
                    cen_in[:, :],
                    idx_t[:, j * (CHUNK_IDX // 16):(j + 1) * (CHUNK_IDX // 16)],
                    CHUNK_IDX,
                    CHUNK_IDX,
                    D,
                    queue_num=j % 4,
                )

                et = e_pool.tile([P, CHUNK_F], mybir.dt.float32, tag="et")
                nc.vector.tensor_sub(et[:], yt[:], gt[:])

                sqt = sq_pool.tile([P, CHUNK_F], mybir.dt.float32, tag="sqt")
                nc.scalar.activation(
                    sqt[:], et[:], mybir.ActivationFunctionType.Square,
                    accum_out=partials[:, j:j + 1],
                )

            colsum = io_pool.tile([P, 1], mybir.dt.float32)
            nc.vector.tensor_reduce(
                colsum[:], partials[:], axis=mybir.AxisListType.X,
                op=mybir.AluOpType.add,
            )
            allred = io_pool.tile([P, 1], mybir.dt.float32)
            nc.gpsimd.partition_all_reduce(
                allred[:], colsum[:], P, bass_isa.ReduceOp.add,
            )
            nc.sync.dma_start(out[:, :], allred[0:1, 0:1])

    nc.compile()
    return nc


def _get_nc():
    if "nc" not in _CACHE:
        _CACHE["nc"] = _build()
    return _CACHE["nc"]


def _make_idx_tile(labels_shard: np.ndarray) -> np.ndarray:
    """int16 SBUF index tile [128, BSH/16] for dma_gather.

    Gather position i writes to SBUF [i%128, i//128]; we want position
    i = t*128 + p to hold centers[labels[p*RPP + t]] so it aligns with the
    y layout (partition p holds rows p*RPP..p*RPP+RPP-1).  The HW reads
    index for position i at [i%16, i//16] of the idx tile (first 16
    partitions, replicated x8 across the Q7 cores).
    """
    lab = np.asarray(labels_shard).astype(np.int16).reshape(P, RPP)
    idx_global = lab.T.reshape(-1)                    # [t*128+p] = labels[p*RPP+t]
    tile16 = idx_global.reshape(BSH // 16, 16).T      # [q, s] = idx_global[s*16+q]
    return np.ascontiguousarray(np.tile(tile16, (P // 16, 1)))


def kernel(y, labels, centers, loss_weight):
    global LAST_RESULTS
    from concourse.bass_utils import run_bass_kernel_spmd

    y = np.asarray(y, dtype=np.float32)
    labels = np.asarray(labels)
    centers = np.ascontiguousarray(np.asarray(centers, dtype=np.float32))

    nc = _get_nc()

    in_maps = []
    for c in range(NCORES):
        sl = slice(c * BSH, (c + 1) * BSH)
        in_maps.append({
            "y": np.ascontiguousarray(y[sl]),
            "idx": _make_idx_tile(labels[sl]),
            "centers": centers,
        })

    res = run_bass_kernel_spmd(
        nc, in_maps, core_ids=list(range(NCORES)), trace=TRACE,
    )
    LAST_RESULTS = res

    total = sum(float(r["out"][0, 0]) for r in res.results)
    total += B * (C - 1) * 1e-12
    loss = total / B * float(np.asarray(loss_weight))
    return np.float32(loss)


# revision 9
# speedup vs baseline: 1.4674x; 1.2196x over previous
"""CenterLoss Trainium2 kernel (Bass/Tile, 8 NeuronCores, data-parallel).

loss = (sum_b clip(||y_b - centers[labels_b]||^2, 1e-12, 1e12)
        + B*(C-1)*1e-12) / B * loss_weight

The masked distmat in the reference reduces to a per-row lookup; off-label
entries of distmat*mask are exactly 0.0 and clip to 1e-12 each (closed-form
constant).  Expanding the square and aggregating by class removes any need
for a per-row gather (GPSIMD gathers cost ~9ns/row of Q7 descriptor
generation plus a ~12us library load):

  sum_b ||y_b - c_{l_b}||^2
    = sum_b ||y_b||^2  +  sum_c n_c ||c_c||^2  -  2 sum_{c,d} S[c,d] centers[c,d]

with n_c = |{b : l_b = c}| (host-side bincount of the integer labels) and
S = onehot^T y computed on the TensorEngine:  S^T[d, c] accumulated over 32
k-tiles of 128 rows, lhsT = y16 k-tile [128b, 128d], rhs = onehot k-tile
[128b, 1024c] (fp16; exact 0/1).  One-hots are built on DVE by comparing an
iota row against the per-partition label (exact in fp16: all values < 2048).
||y||^2 runs on ACT in fp32 (exact); only the zero-mean cross term goes
through fp16, so the end-to-end error stays ~1e-5.

Per-core layout: y as [128, 32*128], partition p holds rows p*32..p*32+31;
k-tile k = free columns [k*128,(k+1)*128) = rows {p*32+k}.
"""

import numpy as np

B = 32768
D = 128
C = 1000
CPAD = 1024                  # classes padded to 2 PSUM banks of fp32
NCORES = 8
BSH = B // NCORES            # 4096 rows per core
P = 128                      # SBUF partitions
RPP = BSH // P               # 32 rows per partition = # k-tiles
NCH = 4                      # y DMA chunks
CHUNK_F = (RPP // NCH) * D   # free elems per y chunk

_CACHE = {}
TRACE = False                # test.py may set kernel.TRACE = True
LAST_RESULTS = None          # BassKernelResults of the last run


def _build():
    import concourse.bacc as bacc
    import concourse.mybir as mybir
    import concourse.tile as tile

    f32 = mybir.dt.float32
    f16 = mybir.dt.float16

    nc = bacc.Bacc("TRN2", target_bir_lowering=False, debug=False,
                   enable_partition_id=False)

    y_in = nc.dram_tensor("y", [BSH, D], f32, kind="ExternalInput")
    lab_in = nc.dram_tensor("lab16", [P, RPP], f32, kind="ExternalInput")
    n_in = nc.dram_tensor("nvec", [P, CPAD // P], f32, kind="ExternalInput")
    cen_in = nc.dram_tensor("centers", [C, D], f32, kind="ExternalInput")
    iota_in = nc.dram_tensor("iota16", [P, CPAD], f16, kind="ExternalInput")
    cent_in = nc.dram_tensor("centersT", [P, C], f32, kind="ExternalInput")
    out = nc.dram_tensor("out", [1, 1], f32, kind="ExternalOutput")

    y_view = y_in.ap().rearrange("(p r) d -> p (r d)", p=P)
    KT = CPAD // P           # 8 center row-tiles / c-tiles

    with tile.TileContext(nc) as tc:
        with (
            tc.tile_pool(name="io", bufs=1) as io_pool,
            tc.tile_pool(name="yb", bufs=4) as y_pool,
            tc.tile_pool(name="oh", bufs=6) as oh_pool,
            tc.tile_pool(name="sc", bufs=2) as sc_pool,
            tc.tile_pool(name="ps", bufs=1, space="PSUM") as psum_pool,
        ):
            lab_t = io_pool.tile([P, RPP], f32)
            nc.sync.dma_start(lab_t[:], lab_in[:, :])
            n_t = io_pool.tile([P, CPAD // P], f32)
            nc.sync.dma_start(n_t[:], n_in[:, :])
            iota_t = io_pool.tile([P, CPAD], f16)
            nc.sync.dma_start(iota_t[:], iota_in[:, :])
            ctsb = io_pool.tile([P, C], f32)
            nc.sync.dma_start(ctsb[:], cent_in[:, :])
            cen_t = io_pool.tile([P, KT * D], f32)
            for k in range(KT):
                lo = k * P
                hi = min(C, lo + P)
                nc.sync.dma_start(cen_t[0:hi - lo, k * D:(k + 1) * D],
                                  cen_in[lo:hi, :])

            y16 = io_pool.tile([P, RPP * D], f16)
            yq = io_pool.tile([P, NCH], f32)
            for j in range(NCH):
                yt = y_pool.tile([P, CHUNK_F], f32, tag="yt")
                nc.sync.dma_start(yt[:], y_view[:, j * CHUNK_F:(j + 1) * CHUNK_F])
                nc.vector.tensor_copy(y16[:, j * CHUNK_F:(j + 1) * CHUNK_F], yt[:])
                sqy = sc_pool.tile([P, CHUNK_F], f32, tag="sqy")
                nc.scalar.activation(
                    sqy[:], yt[:], mybir.ActivationFunctionType.Square,
                    accum_out=yq[:, j:j + 1],
                )

            # q_c = ||c_c||^2 on ACT (fp32, exact)
            qcols = io_pool.tile([P, KT], f32)
            nc.vector.memset(qcols[:], 0.0)
            for k in range(KT):
                lo = k * P
                n_rows = min(C, lo + P) - lo
                sqc = sc_pool.tile([P, D], f32, tag="sqc")
                nc.scalar.activation(
                    sqc[0:n_rows, :], cen_t[0:n_rows, k * D:(k + 1) * D],
                    mybir.ActivationFunctionType.Square,
                    accum_out=qcols[0:n_rows, k:k + 1],
                )

            # S^T[d, c] = sum_b y16[b, d] * onehot[b, c] over 32 k-tiles
            sps = psum_pool.tile([P, CPAD], f32, tag="sps")
            H = CPAD // 2
            for k in range(RPP):
                oh = oh_pool.tile([P, CPAD], f16, tag="oh")
                nc.vector.tensor_scalar(
                    oh[:], iota_t[:], lab_t[:, k:k + 1], None,
                    mybir.AluOpType.is_equal,
                )
                lhsT = y16[:, k * D:(k + 1) * D]
                for h in range(2):
                    nc.tensor.matmul(
                        sps[:, h * H:(h + 1) * H],
                        lhsT,
                        oh[:, h * H:(h + 1) * H],
                        start=(k == 0),
                        stop=(k == RPP - 1),
                    )

            # cross partial = sum_c S^T[:, c] * centersT[:, c]
            scr = io_pool.tile([P, C], f32)
            nc.vector.tensor_mul(scr[:], sps[:, 0:C], ctsb[:])
            crossp = io_pool.tile([P, 1], f32)
            nc.vector.tensor_reduce(
                crossp[:], scr[:], axis=mybir.AxisListType.X,
                op=mybir.AluOpType.add,
            )
            crossm2 = io_pool.tile([P, 1], f32)
            nc.vector.tensor_scalar_mul(crossm2[:], crossp[:], -2.0)
            # term2 partial = sum_k n[:, k] * q[:, k]
            scr2 = io_pool.tile([P, KT], f32)
            nc.vector.tensor_mul(scr2[:], n_t[:], qcols[:])
            t2p = io_pool.tile([P, 1], f32)
            nc.vector.tensor_reduce(
                t2p[:], scr2[:], axis=mybir.AxisListType.X,
                op=mybir.AluOpType.add,
            )
            # term1 partial = sum_j yq[:, j]
            yqcol = io_pool.tile([P, 1], f32)
            nc.vector.tensor_reduce(
                yqcol[:], yq[:], axis=mybir.AxisListType.X,
                op=mybir.AluOpType.add,
            )
            fin = io_pool.tile([P, 1], f32)
            nc.vector.tensor_add(fin[:], yqcol[:], crossm2[:])
            nc.vector.tensor_add(fin[:], fin[:], t2p[:])

            ones = io_pool.tile([P, 1], f32)
            nc.vector.memset(ones[:], 1.0)
            ps = psum_pool.tile([1, 1], f32, tag="fps")
            nc.tensor.matmul(ps[:], fin[:], ones[:])
            res = io_pool.tile([1, 1], f32)
            nc.vector.tensor_copy(res[:], ps[:])
            nc.sync.dma_start(out[:, :], res[0:1, 0:1])

    nc.compile()
    return nc


def _get_nc():
    if "nc" not in _CACHE:
        _CACHE["nc"] = _build()
    return _CACHE["nc"]


_CONST = {}


def _consts():
    if not _CONST:
        _CONST["iota16"] = np.ascontiguousarray(
            np.tile(np.arange(CPAD, dtype=np.float16), (P, 1)))
    return _CONST


def kernel(y, labels, centers, loss_weight):
    global LAST_RESULTS
    from concourse.bass_utils import run_bass_kernel_spmd

    y = np.asarray(y, dtype=np.float32)
    labels = np.asarray(labels).astype(np.int64)
    centers = np.ascontiguousarray(np.asarray(centers, dtype=np.float32))
    centersT = np.ascontiguousarray(centers.T)
    consts = _consts()

    nc = _get_nc()

    in_maps = []
    for c in range(NCORES):
        sl = slice(c * BSH, (c + 1) * BSH)
        lab = labels[sl]
        nvec = np.bincount(lab, minlength=CPAD).astype(np.float32)
        in_maps.append({
            "y": np.ascontiguousarray(y[sl]),
            "lab16": np.ascontiguousarray(
                lab.astype(np.float32).reshape(P, RPP)),
            "nvec": np.ascontiguousarray(
                nvec.reshape(CPAD // P, P).T),
            "centers": centers,
            "centersT": centersT,
            "iota16": consts["iota16"],
        })

    res = run_bass_kernel_spmd(
        nc, in_maps, core_ids=list(range(NCORES)), trace=TRACE,
    )
    LAST_RESULTS = res

    total = sum(float(r["out"][0, 0]) for r in res.results)
    total += B * (C - 1) * 1e-12
    loss = total / B * float(np.asarray(loss_weight))
    return np.float32(loss)
